# revision 2
# baseline (speedup 1.0000x reference)
"""TRN2 kernel for nn_Basic_RT_57750130262767 (gnn_message_passing).

Strategy (per sharding_hint): shard the N (query-node) axis across the 8
NeuronCores. Phase 1: each core computes attention + node update for its 16
query nodes (bank1 sliced over that axis is the dominant HBM traffic,
16MB/core). The small updated node tensor (128KB) is gathered on host.
Phase 2: each core computes its shard of the edge update with the full node
tensor replicated. Bank/graph/param tensors are replicated.

Self-contained: hardcodes all shapes; accepts FULL inputs, returns FULL
(node, edge) outputs.
"""
import numpy as np
import jax
import jax.numpy as jnp
from jax.sharding import Mesh, PartitionSpec as P
from jax.experimental.shard_map import shard_map

B, N, NS, ES, H, HS = 2, 128, 128, 128, 8, 16
NHS, EHS1, EHS2, BOUND = 512, 256, 256, 8
SCALE = 1.0 / float(np.sqrt(HS))
EPS = 1e-5
NCORES = 8
NL = N // NCORES  # 16 query nodes per core

_cache = {}


def _lin(x, p, name):
    return x @ p[name + "_w"] + p[name + "_b"]


def _ln(x, p, name):
    m = jnp.mean(x, axis=-1, keepdims=True)
    v = jnp.mean(jnp.square(x - m), axis=-1, keepdims=True)
    return (x - m) * jax.lax.rsqrt(v + EPS) * p[name + "_s"] + p[name + "_o"]


def _phase1(node_i, edge_i, graph, bank0, bank1_i, p):
    """Attention + node update for this core's NL query nodes."""
    eQ = _lin(edge_i, p, "Weq").reshape(B, NL, N, H, HS).transpose(0, 3, 1, 2, 4)
    eK = _lin(bank1_i, p, "Wek").reshape(BOUND, B, NL, N, H, HS).transpose(0, 1, 4, 2, 3, 5)
    eV = _lin(bank1_i, p, "Wev").reshape(BOUND, B, NL, N, H, HS).transpose(0, 1, 4, 2, 3, 5)
    nQ = _lin(node_i, p, "Wnq").reshape(B, NL, H, HS).transpose(0, 2, 1, 3)
    nK = _lin(bank0, p, "Wnk").reshape(BOUND, B, N, H, HS).transpose(0, 1, 3, 2, 4)
    nV = _lin(bank0, p, "Wnv").reshape(BOUND, B, N, H, HS).transpose(0, 1, 3, 2, 4)
    gQ = _lin(graph, p, "Wgq").reshape(B, H, HS)
    gK = _lin(graph, p, "Wgk").reshape(B, H, HS)
    gV = _lin(graph, p, "Wgv").reshape(B, H, HS)
    Q = eQ + nQ[:, :, :, None, :] + gQ[:, :, None, None, :]          # (B,H,NL,N,HS)
    K = eK + nK[:, :, :, None, :, :] + gK[None, :, :, None, None, :]  # (c,B,H,NL,N,HS)
    V = eV + nV[:, :, :, None, :, :] + gV[None, :, :, None, None, :]
    scores = jnp.einsum("bhijd,cbhijd->bhijc", Q, K) * SCALE          # (B,H,NL,N,c)
    att = jax.nn.softmax(scores.reshape(B, H, NL, N * BOUND), axis=-1)
    att = att.reshape(B, H, NL, N, BOUND)
    out = jnp.einsum("bhijc,cbhijd->bhid", att, V)                    # (B,H,NL,HS)
    attw = out.transpose(0, 2, 1, 3).reshape(B, NL, NS)

    nd = _ln(node_i + _lin(attw, p, "NL1"), p, "NLN1")
    nd = _ln(nd + _lin(jax.nn.relu(_lin(nd, p, "NL2")), p, "NL3"), p, "NLN2")
    return nd


def _phase2(node_full, node_i, edge_i, edge_t, graph, p):
    """Edge update for this core's NL query-node rows."""
    src = jnp.broadcast_to(node_full[:, None, :, :], (B, NL, N, NS))
    tgt = jnp.broadcast_to(node_i[:, :, None, :], (B, NL, N, NS))
    glb = jnp.broadcast_to(graph[:, None, None, :], (B, NL, N, NS))
    edge_t_sw = jnp.swapaxes(edge_t, 1, 2)                            # (B,NL,N,ES)
    cat = jnp.concatenate([edge_i, edge_t_sw, src, tgt, glb], axis=-1)
    ed = _ln(edge_i + _lin(jax.nn.relu(_lin(cat, p, "EL1")), p, "EL2"), p, "ELN1")
    ed = _ln(ed + _lin(jax.nn.relu(_lin(ed, p, "EL3")), p, "EL4"), p, "ELN2")
    return ed


def _build():
    devices = jax.devices()[:NCORES]
    mesh = Mesh(np.asarray(devices), ("x",))
    f1 = shard_map(
        _phase1, mesh=mesh,
        in_specs=(P(None, "x"), P(None, "x"), P(), P(), P(None, None, "x"), P()),
        out_specs=P(None, "x"), check_rep=False)
    f2 = shard_map(
        _phase2, mesh=mesh,
        in_specs=(P(), P(None, "x"), P(None, "x"), P(None, None, "x"), P(), P()),
        out_specs=P(None, "x"), check_rep=False)
    return jax.jit(f1), jax.jit(f2)


def kernel(node_tensors, edge_tensors, graph_tensors, adj_mat, hidden,
           bank0, bank1, step, params):
    if "f1" not in _cache:
        _cache["f1"], _cache["f2"] = _build()
    f1, f2 = _cache["f1"], _cache["f2"]
    node = jnp.asarray(node_tensors, jnp.float32)
    edge = jnp.asarray(edge_tensors, jnp.float32)
    graph = jnp.asarray(graph_tensors, jnp.float32)
    b0 = jnp.asarray(bank0, jnp.float32)
    b1 = jnp.asarray(bank1, jnp.float32)
    p = {k: jnp.asarray(v, jnp.float32) for k, v in params.items()}
    with jax.default_matmul_precision("highest"):
        nd = f1(node, edge, graph, b0, b1, p)
        nd_np = np.asarray(nd, np.float32)          # host gather (128 KB)
        ed = f2(jnp.asarray(nd_np), nd, edge, edge, graph, p)
    return (nd_np, np.asarray(ed, np.float32))
